# revision 18
# baseline (speedup 1.0000x reference)
"""Trainium2 Bass kernel for a 4-layer post-LN transformer decoder.

Model: B=2, T=2048, D=768, 12 heads (dk=64), FF=3072, causal attention,
softmax, post-LN residual blocks, 4 layers.

Sharding (8 cores, SPMD):
  - 2 batch groups of 4 cores: cores 0-3 <-> batch 0, cores 4-7 <-> batch 1.
  - Hidden state h is token-sharded: group-rank r owns tokens [512r, 512r+512),
    stored feature-major [768, 512] f32.
  - Per layer:
      1. qkv for ALL heads of MY tokens computed locally (full qkv weights),
         output features ordered block-major (block j = q|k|v of rank j's
         3 heads, 576 rows).
      2. One ReduceScatter emulates an AllToAll: each rank writes its qkv
         block j into a zero-padded slot (row sub-offset = 576*rank inside
         block j, via a runtime-rank dynamic DMA); RS-sum delivers to rank j
         its 3 heads' q,k,v over ALL 2048 tokens ([576, 2048] bf16).
      3. Flash-style causal attention for my 3 heads over all T.
      4. Partial out-projection over my 192 ctx rows for ALL tokens
         ([768, 2048], out-bias/4 folded in via a constant-1 ctx row), then
         ReduceScatter sums the 4 partials and hands each rank its own
         512-token slice.
      5. residual + LN1 + FFN + residual + LN2 token-locally.
  - Matmuls run in bf16; the residual stream, LN and softmax stats stay f32.
    LN rstd = sqrt(reciprocal(var+eps)) (DVE+Sqrt) to avoid Ln<->Exp
    activation-table swaps.
"""

from contextlib import ExitStack

import numpy as np
import ml_dtypes

import concourse.bass as bass
import concourse.bacc as bacc
import concourse.mybir as mybir
import concourse.tile as tile
from concourse import masks as cmasks
from concourse.bass_utils import run_bass_kernel_spmd

F32 = mybir.dt.float32
BF16 = mybir.dt.bfloat16

B, T, D, DEPTH, HEADS, DK, FF = 2, 2048, 768, 4, 12, 64, 3072
NCORES = 8
G = 4                 # cores per batch group
TOK = T // G          # 512 tokens per core
HPC = HEADS // G      # 3 heads per core
KC = D // 128         # 6 feature chunks
FC = FF // 128        # 24 ff chunks
NSTRIP = T // 512     # 4 token strips per batch
QKVC = 3 * D          # 2304 qkv output features
BLK = QKVC // G       # 576 rows per a2a block
EPS = 1e-5
GROUPS = [[0, 1, 2, 3], [4, 5, 6, 7]]

AF = mybir.ActivationFunctionType
ALU = mybir.AluOpType

# qkv output-feature chunks: per block j (4), 5 chunks each. Block rows are
# 576 = 4.5*128, so chunk boundaries alternate so that every chunk lands
# 128-aligned (or exactly half-aligned) in the GLOBAL (4*576) row space:
# even j: sizes [128,128,128,128,64]; odd j: [64,128,128,128,128].
QKV_CHUNKS = []
for _j in range(G):
    _offs = ((0, 128), (128, 128), (256, 128), (384, 128), (512, 64)) \
        if _j % 2 == 0 else ((0, 64), (64, 128), (192, 128), (320, 128),
                             (448, 128))
    for _cc, (_bo, _sz) in enumerate(_offs):
        QKV_CHUNKS.append((_j, _cc, BLK * _j + _bo, _sz))


def _mm(nc, out, lhsT, rhs, start, stop):
    nc.tensor.matmul(out, lhsT=lhsT, rhs=rhs, start=start, stop=stop)


def build_nc(mode="full"):
    nc = bacc.Bacc("TRN2", target_bir_lowering=False, debug=False,
                   num_devices=NCORES)

    # ---- DRAM parameters (per-core, host-prepared) ----
    xT = nc.declare_dram_parameter("xT", [KC, 128, TOK], F32, isOutput=False)
    peT = nc.declare_dram_parameter("peT", [KC, 128, TOK], F32, isOutput=False)
    qkvw = nc.declare_dram_parameter("qkvw", [DEPTH, KC, 128, QKVC], BF16,
                                     isOutput=False)
    qkvb = nc.declare_dram_parameter("qkvb", [DEPTH, 20, 128], F32, isOutput=False)
    owm = nc.declare_dram_parameter("owm", [DEPTH, 2, 128, D], BF16, isOutput=False)
    f1w = nc.declare_dram_parameter("f1w", [DEPTH, FC, KC, 128, 128], BF16,
                                    isOutput=False)
    f1b = nc.declare_dram_parameter("f1b", [DEPTH, FC, 128], F32, isOutput=False)
    f2w = nc.declare_dram_parameter("f2w", [DEPTH, FC, 128, D], BF16, isOutput=False)
    f2b = nc.declare_dram_parameter("f2b", [DEPTH, D], BF16, isOutput=False)
    ln_g = nc.declare_dram_parameter("ln_g", [DEPTH, 2, KC, 128], F32, isOutput=False)
    ln_b = nc.declare_dram_parameter("ln_b", [DEPTH, 2, KC, 128], F32, isOutput=False)
    outT = nc.declare_dram_parameter("outT", [KC, 128, TOK], F32, isOutput=True)

    with tile.TileContext(nc) as tc, ExitStack() as ctx:
        _build_body(nc, tc, dict(locals(), ctx=ctx, mode=mode))

    if not nc.is_finalized():
        nc.finalize()
    return nc


def _build_body(nc, tc, P):
    xT, peT, qkvw, qkvb, owm = P["xT"], P["peT"], P["qkvw"], P["qkvb"], P["owm"]
    f1w, f1b, f2w, f2b, ln_g, ln_b, outT = (P["f1w"], P["f1b"], P["f2w"],
                                            P["f2b"], P["ln_g"], P["ln_b"],
                                            P["outT"])
    ctx = P["ctx"]
    const = ctx.enter_context(tc.tile_pool(name="const", bufs=1))
    hpool = ctx.enter_context(tc.tile_pool(name="hpool", bufs=2))
    prepool = ctx.enter_context(tc.tile_pool(name="prepool", bufs=1))
    wq = ctx.enter_context(tc.tile_pool(name="wq", bufs=1))
    parm = ctx.enter_context(tc.tile_pool(name="parm", bufs=2))
    bfpool = ctx.enter_context(tc.tile_pool(name="bfpool", bufs=2))
    atn = ctx.enter_context(tc.tile_pool(name="atn", bufs=1))
    strp = ctx.enter_context(tc.tile_pool(name="strp", bufs=4))
    work = ctx.enter_context(tc.tile_pool(name="work", bufs=2))
    opool = ctx.enter_context(tc.tile_pool(name="opool", bufs=2))
    dram = ctx.enter_context(tc.tile_pool(name="dram", bufs=2, space="DRAM"))
    cpool = ctx.enter_context(tc.tile_pool(name="cpool", bufs=1, space="DRAM"))

    # ---- constants ----
    ones_col = const.tile([128, 1], F32)          # LN column-sum lhsT
    nc.vector.memset(ones_col, 1.0)
    ones_row = const.tile([1, 128], F32)          # LN broadcast lhsT
    nc.vector.memset(ones_row, 1.0)
    ones_row_bf = const.tile([1, 512], BF16)      # bias-matmul rhs
    nc.vector.memset(ones_row_bf, 1.0)
    ones65 = const.tile([65, 128], F32)           # denom broadcast lhsT (row 64)
    nc.vector.memset(ones65, 1.0)
    ident = const.tile([128, 128], BF16)          # transpose identity
    cmasks.make_identity(nc, ident[:, :])
    # causal mask patterns for diagonal tiles: keep iff col - row + base >= 0
    amask = const.tile([128, 4, 512], BF16)
    nc.vector.memset(amask, 1.0)
    for dd in range(4):
        nc.gpsimd.affine_select(
            out=amask[:, dd, :], in_=amask[:, dd, :],
            compare_op=ALU.is_ge, fill=0.0, base=-128 * dd,
            channel_multiplier=-1, pattern=[[1, 512]])
    zrow = const.tile([128, TOK], BF16)
    nc.vector.memset(zrow, 0.0)

    # ---- persistent attention tiles ----
    q_sb = atn.tile([128, 2, T], BF16, name="q_sb")
    k_sb = atn.tile([128, 2, T], BF16, name="k_sb")
    v_fm = atn.tile([128, 2, T], BF16, name="v_fm")
    v_sb = atn.tile([128, T // 128, HPC, 65], BF16, name="v_sb")
    nc.vector.memset(v_sb[:, :, :, 64:65], 1.0)
    ctx_sb = atn.tile([128, 2, T], BF16, name="ctx_sb")
    nc.vector.memset(ctx_sb[64:128, 1, :], 0.0)
    nc.vector.memset(ctx_sb[64:65, 1, :], 1.0)   # const-1 row: out-bias matmul

    # ---- persistent collective input buffers (zero-padded), dbl-buffered ----
    # zeroed via the Activation-engine DGE queue to keep the sync queue free
    cins = [cpool.tile([G * BLK, T], BF16, name=f"cin{i}") for i in range(2)]
    for cin in cins:
        zv = cin[:, :].rearrange("(a p) (s n) -> a p s n", p=128, n=TOK)
        for a in range(G * BLK // 128):
            for s2 in range(G):
                nc.scalar.dma_start(out=zv[a, :, s2, :], in_=zrow)

    # ---- h0 = (x + pe)^T (bf16 trunk) ----
    h = hpool.tile([128, KC, TOK], BF16, name="h")
    x_s = prepool.tile([128, KC, TOK], F32, name="pre")
    pe_s = opool.tile([128, KC, TOK], F32, name="pe_s", bufs=1)
    nc.sync.dma_start(out=x_s, in_=xT.ap().rearrange("c p n -> p c n"))
    nc.sync.dma_start(out=pe_s, in_=peT.ap().rearrange("c p n -> p c n"))
    nc.vector.tensor_add(out=h, in0=x_s, in1=pe_s)

    dyn_sem = nc.alloc_semaphore("dyn_sem")
    # runtime group-rank register (used by the per-layer dynamic cin writes)
    with tc.tile_critical():
        rk = nc.gpsimd.alloc_register("rk")
        nc.gpsimd.reg_load(rk, nc.partition_id_tensor[0:1, 0:1])
        nc.gpsimd.reg_alu(rk, rk, 3, ALU.bitwise_and)
        rank = nc.gpsimd.snap(rk, min_val=0, max_val=3)

    for l in range(DEPTH):
        cin = cins[l % 2]
        # ---- per-layer weight loads ----
        qkvw_s = wq.tile([128, KC, QKVC], BF16, name="qkvw_s")
        nc.sync.dma_start(out=qkvw_s, in_=qkvw.ap()[l].rearrange("c p n -> p c n"))
        qkvb_s = parm.tile([128, 20], F32, name="qkvb_s")
        nc.sync.dma_start(out=qkvb_s, in_=qkvb.ap()[l].rearrange("m p -> p m"))
        ow_s = parm.tile([128, 2, D], BF16, name="ow_s")
        nc.sync.dma_start(out=ow_s, in_=owm.ap()[l].rearrange("k p n -> p k n"))
        f1b_s = parm.tile([128, FC], F32, name="f1b_s")
        nc.sync.dma_start(out=f1b_s, in_=f1b.ap()[l].rearrange("k p -> p k"))
        f2b_s = parm.tile([1, D], BF16, name="f2b_s")
        nc.sync.dma_start(out=f2b_s, in_=f2b.ap()[l][None, :])
        g1_s = parm.tile([128, KC], F32, name="g1_s")
        nc.sync.dma_start(out=g1_s, in_=ln_g.ap()[l, 0].rearrange("c p -> p c"))
        b1_s = parm.tile([128, KC], F32, name="b1_s")
        nc.sync.dma_start(out=b1_s, in_=ln_b.ap()[l, 0].rearrange("c p -> p c"))
        g2_s = parm.tile([128, KC], F32, name="g2_s")
        nc.sync.dma_start(out=g2_s, in_=ln_g.ap()[l, 1].rearrange("c p -> p c"))
        b2_s = parm.tile([128, KC], F32, name="b2_s")
        nc.sync.dma_start(out=b2_s, in_=ln_b.ap()[l, 1].rearrange("c p -> p c"))

        # ---- local qkv: all heads, my 512 tokens; per completed block j,
        # dyn-write it into cin (column slot = 512*rank) ----
        qkv_sh = bfpool.tile([128, G, 5, 1, TOK], BF16, name="qkv_sh", bufs=1)
        cvp = cin[:, :].rearrange("(a p) (s n) -> p a s n", p=128, n=TOK)
        with tc.tile_pool(name="qps", bufs=2, space="PSUM") as qps:
            for mi, (j, cc, col0, sz) in enumerate(QKV_CHUNKS):
                ps = qps.tile([128, TOK], F32, name="q_ps")
                for c in range(KC):
                    _mm(nc, ps[0:sz, :], qkvw_s[:, c, col0:col0 + sz],
                        h[:, c, :], c == 0, c == KC - 1)
                nc.vector.tensor_scalar(
                    out=qkv_sh[0:sz, j, cc, 0, :], in0=ps[0:sz, :],
                    scalar1=qkvb_s[0:sz, mi:mi + 1], scalar2=None, op0=ALU.add)

        # ---- RS (emulated A2A): my 3 heads' q,k,v over all T ----
        qkv_all = dram.tile([BLK, T], BF16, name="qkv_all")
        with tc.tile_critical():
            for j in range(G):
                a0 = (BLK * j) // 128          # 0, 4(+half), 9, 13(+half)
                if j % 2 == 0:
                    nc.gpsimd.dma_start(
                        out=cvp[:, a0:a0 + 4, bass.ds(rank, 1), :],
                        in_=qkv_sh[:, j, 0:4, :, :]).then_inc(dyn_sem, 16)
                    nc.gpsimd.dma_start(
                        out=cvp[0:64, a0 + 4:a0 + 5, bass.ds(rank, 1), :],
                        in_=qkv_sh[0:64, j, 4:5, :, :]).then_inc(dyn_sem, 16)
                else:
                    nc.gpsimd.dma_start(
                        out=cvp[64:128, a0:a0 + 1, bass.ds(rank, 1), :],
                        in_=qkv_sh[0:64, j, 0:1, :, :]).then_inc(dyn_sem, 16)
                    nc.gpsimd.dma_start(
                        out=cvp[:, a0 + 1:a0 + 5, bass.ds(rank, 1), :],
                        in_=qkv_sh[:, j, 1:5, :, :]).then_inc(dyn_sem, 16)
            nc.gpsimd.wait_ge(dyn_sem, 128 * (l + 1))
        nc.gpsimd.collective_compute(
            "ReduceScatter", ALU.add, replica_groups=GROUPS,
            ins=[cin[:, :].opt()], outs=[qkv_all[:, :].opt()])

        # ---- attention inputs (strip-split so attention starts early) ----
        for s in range(NSTRIP):
            cols = slice(512 * s, 512 * (s + 1))
            nc.sync.dma_start(out=v_fm[:, 0, cols], in_=qkv_all[384:512, cols])
            nc.sync.dma_start(out=v_fm[0:64, 1, cols], in_=qkv_all[512:576, cols])
            nc.sync.dma_start(out=k_sb[:, 0, cols], in_=qkv_all[192:320, cols])
            nc.sync.dma_start(out=k_sb[0:64, 1, cols], in_=qkv_all[320:384, cols])
            nc.sync.dma_start(out=q_sb[:, 0, cols], in_=qkv_all[0:128, cols])
            nc.sync.dma_start(out=q_sb[0:64, 1, cols], in_=qkv_all[128:192, cols])

        # ---- v transpose: [vf, tok] -> [tok, vf] per 128-token tile ----
        with tc.tile_pool(name="tps", bufs=2, space="PSUM") as tps:
            for tt in range(T // 128):
                pst = tps.tile([128, 192], BF16, name="t_ps")
                nc.tensor.transpose(pst[:, 0:128],
                                    v_fm[:, 0, 128 * tt:128 * (tt + 1)], ident)
                nc.tensor.transpose(pst[:, 128:192],
                                    v_fm[0:64, 1, 128 * tt:128 * (tt + 1)],
                                    ident[0:64, 0:64])
                nc.vector.tensor_copy(
                    out=v_sb[:, tt, :, 0:64],
                    in_=pst.rearrange("p (h d) -> p h d", d=64))

        # ---- causal attention per strip ----
        with (
            tc.tile_pool(name="mmps", bufs=2, space="PSUM") as mmps,
            tc.tile_pool(name="scps", bufs=2, space="PSUM") as scps,
            tc.tile_pool(name="ctxps", bufs=2, space="PSUM") as ctxps,
        ):
            for s in range(NSTRIP):
                nt = 4 * (s + 1)
                q0of = 512 * s

                def _norm(hh, cps):
                    ch, rb = [(0, 0), (0, 64), (1, 0)][hh]
                    den = work.tile([65, 512], F32, name="den", bufs=2)
                    nc.vector.reciprocal(out=den[64:65, :], in_=cps[64:65, :])
                    bc = mmps.tile([128, 512], F32, name="bc_ps", tag="mm")
                    _mm(nc, bc[0:64, :], ones65[64:65, 0:64], den[64:65, :],
                        True, True)
                    bc_sb = work.tile([64, 512], F32, name="bc_sb", bufs=2)
                    nc.vector.tensor_copy(out=bc_sb, in_=bc[0:64, :])
                    nc.vector.tensor_mul(
                        out=ctx_sb[rb:rb + 64, ch, q0of:q0of + 512],
                        in0=cps[0:64, :], in1=bc_sb)

                # pass A: heads 0,1 row-packed
                cps0 = ctxps.tile([65, 512], F32, name="ctx_ps", bufs=2)
                cps1 = ctxps.tile([65, 512], F32, name="ctx_ps", bufs=2)
                for t in range(nt):
                    q0 = 128 * (t - 4 * s) if t >= 4 * s else 0
                    sp = scps.tile([128, 2, 512], F32, name="sc_ps", bufs=2)
                    for hh in range(2):
                        rb = 64 * hh
                        _mm(nc, sp[:, hh, q0:],
                            k_sb[rb:rb + 64, 0, t * 128:(t + 1) * 128],
                            q_sb[rb:rb + 64, 0, q0of + q0:q0of + 512],
                            True, True)
                    pr = work.tile([128, 2, 512], BF16, name="probs", bufs=4)
                    nc.scalar.activation(out=pr[:, :, q0:], in_=sp[:, :, q0:],
                                         func=AF.Exp, scale=0.125)
                    for hh in range(2):
                        if t >= 4 * s:
                            nc.vector.tensor_mul(
                                out=pr[:, hh, q0:], in0=pr[:, hh, q0:],
                                in1=amask[:, t - 4 * s, q0:])
                        _mm(nc, [cps0, cps1][hh][:, q0:], v_sb[:, t, hh, :],
                            pr[:, hh, q0:], t == 0, t == nt - 1)
                _norm(0, cps0)
                _norm(1, cps1)
                # pass B: head 2, two tk-tiles per psum
                cps2 = ctxps.tile([65, 512], F32, name="ctx_ps", bufs=2)
                for tb in range(0, nt, 2):
                    qb = 128 * (tb - 4 * s) if tb >= 4 * s else 0
                    sp = scps.tile([128, 2, 512], F32, name="sc_ps", bufs=2)
                    for jj in range(2):
                        t = tb + jj
                        q0 = 128 * (t - 4 * s) if t >= 4 * s else 0
                        _mm(nc, sp[:, jj, q0:],
                            k_sb[0:64, 1, t * 128:(t + 1) * 128],
                            q_sb[0:64, 1, q0of + q0:q0of + 512], True, True)
                    pr = work.tile([128, 2, 512], BF16, name="probs", bufs=4)
                    nc.scalar.activation(out=pr[:, :, qb:], in_=sp[:, :, qb:],
                                         func=AF.Exp, scale=0.125)
                    for jj in range(2):
                        t = tb + jj
                        q0 = 128 * (t - 4 * s) if t >= 4 * s else 0
                        if t >= 4 * s:
                            nc.vector.tensor_mul(
                                out=pr[:, jj, q0:], in0=pr[:, jj, q0:],
                                in1=amask[:, t - 4 * s, q0:])
                        _mm(nc, cps2[:, q0:], v_sb[:, t, 2, :], pr[:, jj, q0:],
                            t == 0, t == nt - 1)
                _norm(2, cps2)

        # ---- partial out-proj over my ctx rows, all T -> RS ----
        rs2_in = dram.tile([G * D, TOK], BF16, name="rs2_in")
        rs2v = rs2_in[:, :].rearrange("(s c p) n -> s p c n", c=KC, p=128)
        with tc.tile_pool(name="ops", bufs=2, space="PSUM") as ops:
            for s in range(NSTRIP):
                o_sb = opool.tile([128, KC, 512], BF16, name="o_sb")
                for m in range(KC):
                    ps = ops.tile([128, 512], F32, name="op_ps")
                    _mm(nc, ps, ow_s[:, 0, m * 128:(m + 1) * 128],
                        ctx_sb[:, 0, 512 * s:512 * (s + 1)], True, False)
                    _mm(nc, ps, ow_s[:, 1, m * 128:(m + 1) * 128],
                        ctx_sb[:, 1, 512 * s:512 * (s + 1)], False, True)
                    nc.vector.tensor_copy(out=o_sb[:, m, :], in_=ps)
                nc.sync.dma_start(out=rs2v[s], in_=o_sb)
        o_mine = dram.tile([D, TOK], BF16, name="o_mine")
        nc.gpsimd.collective_compute(
            "ReduceScatter", ALU.add, replica_groups=GROUPS,
            ins=[rs2_in[:, :].opt()], outs=[o_mine[:, :].opt()])

        # ---- residual + LN1 -> h1 (bf16) ----
        o_f = bfpool.tile([128, KC, TOK], BF16, name="o_f", bufs=1)
        o_v = o_mine[:, :].rearrange("(c p) n -> p c n", p=128)
        h1pre = prepool.tile([128, KC, TOK], F32, name="pre")
        for c in range(KC):
            nc.sync.dma_start(out=o_f[:, c, :], in_=o_v[:, c, :])
            nc.vector.tensor_add(out=h1pre[:, c, :], in0=h[:, c, :],
                                 in1=o_f[:, c, :])
        h1 = hpool.tile([128, KC, TOK], BF16, name="h")
        _layernorm(nc, tc, h1pre, h1, g1_s, b1_s, ones_col, ones_row, work)

        # ---- FFN (k-pipelined) + residual -> h2pre ----
        h1_bf = h1
        h2pre = prepool.tile([128, KC, TOK], F32, name="pre")
        with (
            tc.tile_pool(name="f2ps", bufs=1, space="PSUM") as f2ps,
            tc.tile_pool(name="f1ps", bufs=2, space="PSUM") as f1ps,
        ):
            accs = [f2ps.tile([128, 512], F32, name=f"f2_ps{m}") for m in range(KC)]
            for k in range(FC):
                w1c = strp.tile([128, KC, 128], BF16, name="w1c", bufs=4)
                nc.sync.dma_start(out=w1c, in_=f1w.ap()[l, k].rearrange("c p n -> p c n"))
                w2r = strp.tile([128, D], BF16, name="w2r", bufs=4)
                nc.sync.dma_start(out=w2r, in_=f2w.ap()[l, k])
                ap = f1ps.tile([128, 512], F32, name="a_ps")
                for c in range(KC):
                    _mm(nc, ap, w1c[:, c, :], h1_bf[:, c, :], c == 0, c == KC - 1)
                a_bf = work.tile([128, 512], BF16, name="a_bf", bufs=2)
                nc.scalar.activation(out=a_bf, in_=ap, func=AF.Relu,
                                     bias=f1b_s[:, k:k + 1], scale=1.0)
                for m in range(KC):
                    _mm(nc, accs[m], w2r[:, m * 128:(m + 1) * 128], a_bf,
                        k == 0, False)
            for m in range(KC):
                _mm(nc, accs[m], f2b_s[:, m * 128:(m + 1) * 128], ones_row_bf,
                    False, True)
                nc.vector.tensor_add(out=h2pre[:, m, :], in0=accs[m],
                                     in1=h1[:, m, :])

        # ---- LN2 -> h (next layer, bf16) ----
        h = hpool.tile([128, KC, TOK], BF16, name="h")
        _layernorm(nc, tc, h2pre, h, g2_s, b2_s, ones_col, ones_row, work)

    h_out = prepool.tile([128, KC, TOK], F32, name="pre")
    for c in range(KC):
        nc.vector.tensor_copy(out=h_out[:, c, :], in_=h[:, c, :])
    nc.sync.dma_start(out=outT.ap().rearrange("c p n -> p c n"), in_=h_out)


def _layernorm(nc, tc, x, out, g_s, b_s, ones_col, ones_row, work):
    """out[:, c, :] = (x - mean)/sqrt(var+eps) * g + b, mean/var over features
    (partition x chunk dims), per token (free dim). x, out: [128, KC, TOK] f32.
    rstd = Sqrt(1/(var+eps)) -- avoids Ln/Exp act-table churn."""
    with tc.tile_pool(name="lnps", bufs=1, space="PSUM") as lnps:
        sq = work.tile([128, 512], F32, name="lnsq", bufs=2)
        s1 = lnps.tile([1, 512], F32, name="s1_ps")
        s2 = lnps.tile([1, 512], F32, name="s2_ps")
        for c in range(KC):
            _mm(nc, s1, ones_col, x[:, c, :], c == 0, c == KC - 1)
        for c in range(KC):
            nc.vector.tensor_mul(out=sq, in0=x[:, c, :], in1=x[:, c, :])
            _mm(nc, s2, ones_col, sq, c == 0, c == KC - 1)
        st = work.tile([1, 3, 512], F32, name="lnst", bufs=1)
        mean = st[:, 0, :]
        nc.vector.tensor_scalar(out=mean, in0=s1, scalar1=1.0 / D, scalar2=None,
                                op0=ALU.mult)
        var = st[:, 1, :]
        nc.vector.tensor_scalar(out=var, in0=s2, scalar1=1.0 / D, scalar2=EPS,
                                op0=ALU.mult, op1=ALU.add)
        m2 = st[:, 2, :]
        nc.vector.tensor_mul(out=m2, in0=mean, in1=mean)
        nc.vector.tensor_tensor(out=var, in0=var, in1=m2, op=ALU.subtract)
        rinv = m2
        nc.vector.reciprocal(out=rinv, in_=var)
        nc.scalar.activation(out=var, in_=rinv, func=AF.Sqrt, bias=0.0, scale=1.0)
        mb = lnps.tile([128, 512], F32, name="mb_ps")
        rb = lnps.tile([128, 512], F32, name="rb_ps")
        _mm(nc, mb, ones_row, mean, True, True)
        _mm(nc, rb, ones_row, var, True, True)
        for c in range(KC):
            t1 = work.tile([128, 512], F32, name="lnt1", bufs=2)
            nc.vector.tensor_tensor(out=t1, in0=x[:, c, :], in1=mb,
                                    op=ALU.subtract)
            nc.vector.tensor_tensor(out=t1, in0=t1, in1=rb, op=ALU.mult)
            nc.vector.tensor_scalar(out=out[:, c, :], in0=t1,
                                    scalar1=g_s[:, c:c + 1],
                                    scalar2=b_s[:, c:c + 1],
                                    op0=ALU.mult, op1=ALU.add)


_NC_CACHE = None


def _get_nc():
    global _NC_CACHE
    if _NC_CACHE is None:
        _NC_CACHE = build_nc("full")
    return _NC_CACHE


def _pos_encoding():
    pos = np.arange(T, dtype=np.float32)[:, None]
    div = np.exp(np.arange(0, D, 2, dtype=np.float32) * (-np.log(10000.0) / D))
    pe = np.zeros((T, D), dtype=np.float32)
    pe[:, 0::2] = np.sin(pos * div)
    pe[:, 1::2] = np.cos(pos * div)
    return pe


def make_in_maps(inputs):
    x = np.asarray(inputs["x"], dtype=np.float32)
    qkv_w = np.asarray(inputs["qkv_w"], dtype=np.float32)
    qkv_b = np.asarray(inputs["qkv_b"], dtype=np.float32)
    out_w = np.asarray(inputs["out_w"], dtype=np.float32)
    out_b = np.asarray(inputs["out_b"], dtype=np.float32)
    ff1_w = np.asarray(inputs["ff1_w"], dtype=np.float32)
    ff1_b = np.asarray(inputs["ff1_b"], dtype=np.float32)
    ff2_w = np.asarray(inputs["ff2_w"], dtype=np.float32)
    ff2_b = np.asarray(inputs["ff2_b"], dtype=np.float32)
    ln1_g = np.asarray(inputs["ln1_g"], dtype=np.float32)
    ln1_b = np.asarray(inputs["ln1_b"], dtype=np.float32)
    ln2_g = np.asarray(inputs["ln2_g"], dtype=np.float32)
    ln2_b = np.asarray(inputs["ln2_b"], dtype=np.float32)
    pe = _pos_encoding()
    bf = ml_dtypes.bfloat16

    # qkv output-feature permutation: block j = [q|k|v for heads 3j..3j+2]
    perm = []
    for j in range(G):
        for base in (0, D, 2 * D):
            for i in range(HPC):
                hh = HPC * j + i
                perm.extend(range(base + DK * hh, base + DK * (hh + 1)))
    perm = np.asarray(perm)
    qkvw_r = qkv_w[:, :, perm]                       # [DEPTH, 768, 2304]
    qkvb_r = qkv_b[:, perm]                          # [DEPTH, 2304]

    qkvw_a = np.ascontiguousarray(
        qkvw_r.reshape(DEPTH, KC, 128, QKVC)).astype(bf)
    qkvb_a = np.zeros((DEPTH, 20, 128), np.float32)
    for mi, (j, cc, col0, sz) in enumerate(QKV_CHUNKS):
        qkvb_a[:, mi, 0:sz] = qkvb_r[:, col0:col0 + sz]

    f1w_a = np.ascontiguousarray(
        ff1_w.reshape(DEPTH, KC, 128, FC, 128).transpose(0, 3, 1, 2, 4)).astype(bf)
    f1b_a = np.ascontiguousarray(ff1_b.reshape(DEPTH, FC, 128))
    f2w_a = np.ascontiguousarray(ff2_w.reshape(DEPTH, FC, 128, D)).astype(bf)
    f2b_a = ff2_b.astype(bf)
    lng_a = np.ascontiguousarray(
        np.stack([ln1_g, ln2_g], axis=1).reshape(DEPTH, 2, KC, 128))
    lnb_a = np.ascontiguousarray(
        np.stack([ln1_b, ln2_b], axis=1).reshape(DEPTH, 2, KC, 128))

    in_maps = []
    for core in range(NCORES):
        b, r = core // G, core % G
        toks = slice(TOK * r, TOK * (r + 1))

        xT_a = np.ascontiguousarray(x[b, toks].T.reshape(KC, 128, TOK))
        peT_a = np.ascontiguousarray(pe[toks].T.reshape(KC, 128, TOK))

        owm_a = np.zeros((DEPTH, 2, 128, D), np.float32)
        owm_a[:, 0, :, :] = out_w[:, 192 * r:192 * r + 128, :]
        owm_a[:, 1, 0:64, :] = out_w[:, 192 * r + 128:192 * r + 192, :]
        owm_a[:, 1, 64, :] = out_b / G

        in_maps.append({
            "xT": xT_a, "peT": peT_a, "qkvw": qkvw_a, "qkvb": qkvb_a,
            "owm": owm_a.astype(bf), "f1w": f1w_a, "f1b": f1b_a,
            "f2w": f2w_a, "f2b": f2b_a, "ln_g": lng_a, "ln_b": lnb_a,
        })
    return in_maps


def kernel(**inputs) -> np.ndarray:
    in_maps = make_in_maps(inputs)
    nc = _get_nc()
    res = run_bass_kernel_spmd(nc, in_maps, core_ids=list(range(NCORES)))
    out = np.zeros((B, T, D), dtype=np.float32)
    for core in range(NCORES):
        b, r = core // G, core % G
        hT = res.results[core]["outT"].reshape(D, TOK)  # [768, 512]
        out[b, TOK * r:TOK * (r + 1), :] = hT.T
    return out
